# revision 1
# baseline (speedup 1.0000x reference)
"""MoE (top-2 of 8 experts + shared expert) Trainium2 Bass kernel.

Strategy (expert-parallel, host-prepped routing):
  - Router (sigmoid gate + top-2) is tiny (0.27 GFLOP) and runs on the host
    in fp32; it also produces the token->expert gather lists.
  - Core c computes expert c's SwiGLU FFN densely over the tokens routed to
    it (padded to the max per-expert count), plus the shared-expert FFN over
    the token shard [c*1024, (c+1)*1024).
  - All matmuls run in float32r (full PE rate, ~1.5e-4 rel err).
  - Host scatter-adds the per-expert outputs (scaled by the combine weights)
    and the shared outputs into the final [8192, 2048] result.

Everything on-device is laid out feature-major ("K on partitions") so the
x @ W.T chains need no on-chip transposes:
  stage1:  h1T[m,:] = sum_k w1T[k, m].T @ xT[k, :]   (PSUM accum over k)
  g = silu(h1T) * h3T                                 (ACT + DVE)
  stage2:  yT[md,:] = sum_kh w2T[kh, md].T @ gT[kh,:]
"""

import os
import sys

for _p in ("/opt/trn_rl_repo", "/root/.axon_site/_ro/trn_rl_repo"):
    if os.path.isdir(_p) and _p not in sys.path:
        sys.path.insert(0, _p)

import numpy as np

import concourse.bass as bass  # noqa: F401
import concourse.mybir as mybir
import concourse.tile as tile
from concourse import bacc
from concourse.bass_utils import run_bass_kernel_spmd

# Problem constants (hardcoded per spec)
N_TOK = 8192
D = 2048
H = 1408
E = 8
TOP_K = 2
ROUTE_SCALE = 1.0
P = 128
KD = D // P    # 16 k-tiles over D
MH = H // P    # 11 m-tiles over H
MD = D // P    # 16 m-tiles over D (stage 2 out)
SHARD = N_TOK // E  # 1024 shared-expert tokens per core

MAX_CHUNK = 1024  # tokens per weight-stream chunk (<=1024 keeps PSUM rotation slack)

F32 = mybir.dt.float32
F32R = mybir.dt.float32r
SILU = mybir.ActivationFunctionType.Silu

LAST_RESULTS = None  # BassKernelResults of the most recent run (for test.py)


def _chunks(T):
    """Split T (multiple of 128) into chunks of <=MAX_CHUNK, multiples of 128."""
    n = -(-T // MAX_CHUNK)
    base = T // n
    sizes = []
    rem = T
    for i in range(n):
        left = n - i
        c = min(MAX_CHUNK, -(-rem // left + 127) // 128 * 128) if left > 1 else rem
        c = min(c, rem)
        sizes.append(c)
        rem -= c
    assert sum(sizes) == T and all(s % 128 == 0 for s in sizes), sizes
    return sizes


def _subs(Tc):
    """Split Tc into matmul free-dim slices, preferring all >=256 (f32r fast)."""
    out = []
    rem = Tc
    while rem > 512:
        take = 384 if rem == 640 else 512
        out.append(take)
        rem -= take
    if rem:
        out.append(rem)
    s0 = 0
    res = []
    for s in out:
        res.append((s0, s))
        s0 += s
    return res


SKIP_MM = bool(os.environ.get("MOE_SKIP_MM"))
SKIP_DMA = bool(os.environ.get("MOE_SKIP_DMA"))


def _emit_ffn(nc, pools, x_dram, w1_dram, w3_dram, w2_dram, y_dram, T):
    """Emit one feature-major SwiGLU FFN over T tokens."""
    xpool, wpool, w2pool, gpool, spool, ypool, psum = pools

    def mm(*a, **k):
        if not SKIP_MM:
            nc.tensor.matmul(*a, **k)

    def dma(*a, **k):
        if not SKIP_DMA:
            nc.sync.dma_start(*a, **k)
    cs = 0
    for Tc in _chunks(T):
        x_tiles = []
        for k in range(KD):
            xt = xpool.tile([P, Tc], F32R, name=f"x{k}")
            SKIP_DMA or nc.sync.dma_start(xt[:], x_dram[k, :, cs:cs + Tc])
            x_tiles.append(xt)
        subs = _subs(Tc)
        g_tiles = []
        for m in range(MH):
            w1m = wpool.tile([P, KD * P], F32R, name="w1m")
            SKIP_DMA or nc.sync.dma_start(w1m[:], w1_dram[m])
            w3m = wpool.tile([P, KD * P], F32R, name="w3m")
            SKIP_DMA or nc.sync.dma_start(w3m[:], w3_dram[m])
            gm = gpool.tile([P, Tc], F32R, name=f"g{m}")
            # weight-stationary: consecutive MMs share one lhsT so the PE
            # skips the per-MM weight reload (measured 258 -> 135 ns/MM)
            ps1 = [psum.tile([P, 512], F32, name="acc")[:, :sl] for _, sl in subs]
            ps3 = [psum.tile([P, 512], F32, name="acc")[:, :sl] for _, sl in subs]
            for k in range(KD):
                for j, (s0, sl) in enumerate(subs):
                    SKIP_MM or nc.tensor.matmul(
                        ps1[j], w1m[:, k * P:(k + 1) * P], x_tiles[k][:, s0:s0 + sl],
                        start=(k == 0), stop=(k == KD - 1),
                    )
                for j, (s0, sl) in enumerate(subs):
                    SKIP_MM or nc.tensor.matmul(
                        ps3[j], w3m[:, k * P:(k + 1) * P], x_tiles[k][:, s0:s0 + sl],
                        start=(k == 0), stop=(k == KD - 1),
                    )
            for j, (s0, sl) in enumerate(subs):
                st = spool.tile([P, 512], F32, name="silu")[:, :sl]
                SKIP_MM or nc.scalar.activation(st, ps1[j], SILU)
                SKIP_MM or nc.vector.tensor_mul(gm[:, s0:s0 + sl], st, ps3[j])
            g_tiles.append(gm)
        for md in range(MD):
            w2m = w2pool.tile([P, MH * P], F32R, name="w2m")
            SKIP_DMA or nc.sync.dma_start(w2m[:], w2_dram[md])
            ym = ypool.tile([P, Tc], F32, name="ym")
            psy = [psum.tile([P, 512], F32, name="acc")[:, :sl] for _, sl in subs]
            for kh in range(MH):
                for j, (s0, sl) in enumerate(subs):
                    SKIP_MM or nc.tensor.matmul(
                        psy[j], w2m[:, kh * P:(kh + 1) * P], g_tiles[kh][:, s0:s0 + sl],
                        start=(kh == 0), stop=(kh == MH - 1),
                    )
            for j, (s0, sl) in enumerate(subs):
                SKIP_MM or nc.vector.tensor_copy(ym[:, s0:s0 + sl], psy[j])
            SKIP_MM or nc.sync.dma_start(y_dram[md, :, cs:cs + Tc], ym[:])
        cs += Tc


def _build_program(c_cap, loop_reps=1):
    nc = bacc.Bacc("TRN2", target_bir_lowering=False, debug=False, num_devices=E)
    xe = nc.dram_tensor("xe", [KD, P, c_cap], F32R, kind="ExternalInput").ap()
    xs = nc.dram_tensor("xs", [KD, P, SHARD], F32R, kind="ExternalInput").ap()
    w1s = nc.dram_tensor("w1s", [MH, P, KD * P], F32R, kind="ExternalInput").ap()
    w3s = nc.dram_tensor("w3s", [MH, P, KD * P], F32R, kind="ExternalInput").ap()
    w2s = nc.dram_tensor("w2s", [MD, P, MH * P], F32R, kind="ExternalInput").ap()
    sw1s = nc.dram_tensor("sw1s", [MH, P, KD * P], F32R, kind="ExternalInput").ap()
    sw3s = nc.dram_tensor("sw3s", [MH, P, KD * P], F32R, kind="ExternalInput").ap()
    sw2s = nc.dram_tensor("sw2s", [MD, P, MH * P], F32R, kind="ExternalInput").ap()
    ye = nc.dram_tensor("ye", [MD, P, c_cap], F32, kind="ExternalOutput").ap()
    ys = nc.dram_tensor("ys", [MD, P, SHARD], F32, kind="ExternalOutput").ap()

    with tile.TileContext(nc) as tc:
        with tc.tile_pool(name="xpool", bufs=1) as xpool, \
             tc.tile_pool(name="wpool", bufs=2) as wpool, \
             tc.tile_pool(name="w2pool", bufs=3) as w2pool, \
             tc.tile_pool(name="gpool", bufs=1) as gpool, \
             tc.tile_pool(name="spool", bufs=3) as spool, \
             tc.tile_pool(name="ypool", bufs=2) as ypool, \
             tc.tile_pool(name="psum", bufs=6, space="PSUM") as psum:
            pools = (xpool, wpool, w2pool, gpool, spool, ypool, psum)
            if loop_reps > 1:
                with tc.For_i(0, loop_reps, 1):
                    _emit_ffn(nc, pools, xe, w1s, w3s, w2s, ye, c_cap)
                    _emit_ffn(nc, pools, xs, sw1s, sw3s, sw2s, ys, SHARD)
            else:
                _emit_ffn(nc, pools, xe, w1s, w3s, w2s, ye, c_cap)
                _emit_ffn(nc, pools, xs, sw1s, sw3s, sw2s, ys, SHARD)
    nc.compile()
    return nc


def _tile_w13(w):
    # [H, D] -> [MH, P, KD*P] with slab[m, p, k*P+j] = w[m*P+j, k*P+p]
    return np.ascontiguousarray(
        w.reshape(MH, P, KD, P).transpose(0, 3, 2, 1).reshape(MH, P, KD * P)
    )


def _tile_w2(w):
    # [D, H] -> [MD, P, MH*P] with slab[md, p, kh*P+j] = w[md*P+j, kh*P+p]
    return np.ascontiguousarray(
        w.reshape(MD, P, MH, P).transpose(0, 3, 2, 1).reshape(MD, P, MH * P)
    )


def _tile_x(xt):
    # [T, D] -> [KD, P, T]
    T = xt.shape[0]
    return np.ascontiguousarray(xt.reshape(T, KD, P).transpose(1, 2, 0))


def _untile_y(y):
    # [MD, P, T] -> [T, D]
    return y.transpose(2, 0, 1).reshape(y.shape[2], D)


def prepare(x, gate_w, expert_bias, w1, w2, w3, sw1, sw2, sw3):
    """Host routing + input prep. Returns (nc, in_maps, meta)."""
    x = np.ascontiguousarray(np.asarray(x, dtype=np.float32))
    gate_w = np.asarray(gate_w, dtype=np.float32)
    expert_bias = np.asarray(expert_bias, dtype=np.float32)
    w1 = np.asarray(w1, dtype=np.float32)
    w2 = np.asarray(w2, dtype=np.float32)
    w3 = np.asarray(w3, dtype=np.float32)
    sw1 = np.asarray(sw1, dtype=np.float32)
    sw2 = np.asarray(sw2, dtype=np.float32)
    sw3 = np.asarray(sw3, dtype=np.float32)

    # ---- host router (fp32, matches reference numerics) ----
    logits = x @ gate_w.T  # [N, E] f32
    scores = np.where(
        logits >= 0,
        1.0 / (1.0 + np.exp(-logits, dtype=np.float32)),
        np.exp(logits, dtype=np.float32) / (1.0 + np.exp(logits, dtype=np.float32)),
    ).astype(np.float32)
    biased = scores + expert_bias[None, :]
    i1 = np.argmax(biased, axis=1)
    tmp = biased.copy()
    tmp[np.arange(N_TOK), i1] = -np.inf
    i2 = np.argmax(tmp, axis=1)
    s1 = scores[np.arange(N_TOK), i1]
    s2 = scores[np.arange(N_TOK), i2]
    denom = s1 + s2 + np.float32(1e-20)
    c1 = (s1 / denom * np.float32(ROUTE_SCALE)).astype(np.float32)
    c2 = (s2 / denom * np.float32(ROUTE_SCALE)).astype(np.float32)

    idx_list, cw_list = [], []
    for e in range(E):
        m1 = i1 == e
        m2 = i2 == e
        idx = np.concatenate([np.nonzero(m1)[0], np.nonzero(m2)[0]])
        cw = np.concatenate([c1[m1], c2[m2]]).astype(np.float32)
        idx_list.append(idx)
        cw_list.append(cw)
    counts = [len(i) for i in idx_list]
    c_cap = max(512, -(-max(counts) // 128) * 128)

    # ---- build + compile the SPMD program for this capacity ----
    nc = _build_program(c_cap, loop_reps=int(os.environ.get("MOE_LOOP_REPS", "1")))

    # ---- per-core inputs ----
    in_maps = []
    sw1s = _tile_w13(sw1)
    sw3s = _tile_w13(sw3)
    sw2s = _tile_w2(sw2)
    for c in range(E):
        idx = idx_list[c]
        pad = c_cap - len(idx)
        idx_pad = np.concatenate([idx, np.zeros(pad, dtype=idx.dtype)]) if pad else idx
        xe = x[idx_pad]
        in_maps.append({
            "xe": _tile_x(xe),
            "xs": _tile_x(x[c * SHARD:(c + 1) * SHARD]),
            "w1s": _tile_w13(w1[c]),
            "w3s": _tile_w13(w3[c]),
            "w2s": _tile_w2(w2[c]),
            "sw1s": sw1s,
            "sw3s": sw3s,
            "sw2s": sw2s,
        })

    meta = (idx_list, cw_list, counts)
    return nc, in_maps, meta


def combine(meta, results):
    """Scatter-add per-core outputs into the final [N, D] array."""
    idx_list, cw_list, counts = meta
    out = np.zeros((N_TOK, D), dtype=np.float32)
    for c in range(E):
        r = results[c]
        cnt = counts[c]
        if cnt:
            y_tok = _untile_y(r["ye"])[:cnt]
            out[idx_list[c]] += cw_list[c][:, None] * y_tok
        out[c * SHARD:(c + 1) * SHARD] += _untile_y(r["ys"])
    return out


def kernel(x, gate_w, expert_bias, w1, w2, w3, sw1, sw2, sw3):
    nc, in_maps, meta = prepare(x, gate_w, expert_bias, w1, w2, w3, sw1, sw2, sw3)
    global LAST_RESULTS
    res = run_bass_kernel_spmd(nc, in_maps, core_ids=list(range(E)))
    LAST_RESULTS = res
    return combine(meta, res.results)



# revision 2
# speedup vs baseline: 1.2018x; 1.2018x over previous
"""MoE (top-2 of 8 experts + shared expert) Trainium2 Bass kernel.

Strategy (expert-parallel, host-prepped routing):
  - Router (sigmoid gate + top-2) runs on the host in fp32 (it is tiny);
    it produces per-expert token gather lists and combine weights.
  - Core c computes expert c's SwiGLU FFN densely over the tokens routed to
    it (padded to the max per-expert count, 64-rounded), plus the shared
    expert FFN over the token shard [c*1024, (c+1)*1024).
  - Host scatter-adds the per-expert outputs (scaled by combine weights)
    and the shared outputs into the final [8192, 2048] result.

Device-side numerics: all matmul operands are bf16 (halves HBM traffic vs
f32/f32r at the same 1 cycle/row PE rate); accumulation is fp32 in PSUM.
Measured end-to-end rel err ~1e-3 vs the 2e-2 gate.

Schedule: feature-major layout ("K on partitions"), weight tiles are the
OUTER loop so each weight byte is DMA'd exactly once; all x tiles stay
resident in SBUF (bf16 makes them fit) and the token dimension is walked
in column chunks sized so each PSUM accumulator is <=512 fp32 (one bank):
  stage1:  ps1/ps3[m, cols] = sum_k w1T/w3T[k, m].T @ xT[k, cols]
  g = silu(ps1) * ps3                      (ACT + DVE, cast to bf16)
  stage2:  yT[md, cols] = sum_kh w2T[kh, md].T @ gT[kh, cols]
"""

import os
import sys

for _p in ("/opt/trn_rl_repo", "/root/.axon_site/_ro/trn_rl_repo"):
    if os.path.isdir(_p) and _p not in sys.path:
        sys.path.insert(0, _p)

import ml_dtypes
import numpy as np

import concourse.bass as bass  # noqa: F401
import concourse.mybir as mybir
import concourse.tile as tile
from concourse import bacc
from concourse.bass_utils import run_bass_kernel_spmd

# Problem constants (hardcoded per spec)
N_TOK = 8192
D = 2048
H = 1408
E = 8
TOP_K = 2
ROUTE_SCALE = 1.0
P = 128
KD = D // P    # 16 k-tiles over D
MH = H // P    # 11 m-tiles over H
MD = D // P    # 16 m-tiles over D (stage 2 out)
SHARD = N_TOK // E  # 1024 shared-expert tokens per core

F32 = mybir.dt.float32
BF16 = mybir.dt.bfloat16
NP_BF16 = ml_dtypes.bfloat16
SILU = mybir.ActivationFunctionType.Silu

LAST_RESULTS = None  # BassKernelResults of the most recent run (for test.py)

SKIP_MM = bool(os.environ.get("MOE_SKIP_MM"))
SKIP_DMA = bool(os.environ.get("MOE_SKIP_DMA"))


def _col_chunks(T):
    """Split T (multiple of 64) into even chunks of <=1024 cols (<=2 PSUM
    banks per accumulator side), multiples of 64."""
    n = -(-T // 1024)
    c = (-(-T // n) + 63) // 64 * 64
    out = []
    c0 = 0
    while c0 < T:
        cw = min(c, T - c0)
        out.append((c0, cw))
        c0 += cw
    return out


def _subs_of(cw):
    """Split a chunk into <=512-col PSUM sub-tiles."""
    out = []
    s0 = 0
    while s0 < cw:
        sl = min(512, cw - s0)
        out.append((s0, sl))
        s0 += sl
    return out


def _emit_stage1(nc, pools, x_tiles, w1_dram, w3_dram, g_tiles, T):
    """g[m] = silu(x @ w1.T) * (x @ w3.T), feature-major, weights outer."""
    wpool, spool, psum = pools
    chunks = _col_chunks(T)
    for m in range(MH):
        w1m = wpool.tile([P, KD * P], BF16, name="w1m")
        SKIP_DMA or nc.sync.dma_start(w1m[:], w1_dram[m])
        w3m = wpool.tile([P, KD * P], BF16, name="w3m")
        SKIP_DMA or nc.sync.dma_start(w3m[:], w3_dram[m])
        for c0, cw in chunks:
            subs = _subs_of(cw)
            ps1 = [psum.tile([P, 512], F32, name="acc")[:, :sl] for _, sl in subs]
            ps3 = [psum.tile([P, 512], F32, name="acc")[:, :sl] for _, sl in subs]
            for k in range(KD):
                for j, (s0, sl) in enumerate(subs):
                    SKIP_MM or nc.tensor.matmul(
                        ps1[j], w1m[:, k * P:(k + 1) * P],
                        x_tiles[k][:, c0 + s0:c0 + s0 + sl],
                        start=(k == 0), stop=(k == KD - 1),
                    )
                for j, (s0, sl) in enumerate(subs):
                    SKIP_MM or nc.tensor.matmul(
                        ps3[j], w3m[:, k * P:(k + 1) * P],
                        x_tiles[k][:, c0 + s0:c0 + s0 + sl],
                        start=(k == 0), stop=(k == KD - 1),
                    )
            for j, (s0, sl) in enumerate(subs):
                st = spool.tile([P, 512], F32, name="silu")[:, :sl]
                SKIP_MM or nc.scalar.activation(st, ps1[j], SILU)
                SKIP_MM or nc.vector.tensor_mul(
                    g_tiles[m][:, c0 + s0:c0 + s0 + sl], st, ps3[j])


def _emit_stage2(nc, pools, g_tiles, w2_dram, y_dram, ypool, T):
    """y[md] = g @ w2.T, feature-major, weights outer."""
    w2pool, psum = pools
    chunks = _col_chunks(T)
    for md in range(MD):
        w2m = w2pool.tile([P, MH * P], BF16, name="w2m")
        SKIP_DMA or nc.sync.dma_start(w2m[:], w2_dram[md])
        ym = ypool.tile([P, T], BF16, name="ym")
        for c0, cw in chunks:
            subs = _subs_of(cw)
            psy = [psum.tile([P, 512], F32, name="acc")[:, :sl] for _, sl in subs]
            for kh in range(MH):
                for j, (s0, sl) in enumerate(subs):
                    SKIP_MM or nc.tensor.matmul(
                        psy[j], w2m[:, kh * P:(kh + 1) * P],
                        g_tiles[kh][:, c0 + s0:c0 + s0 + sl],
                        start=(kh == 0), stop=(kh == MH - 1),
                    )
            for j, (s0, sl) in enumerate(subs):
                SKIP_MM or nc.vector.tensor_copy(ym[:, c0 + s0:c0 + s0 + sl],
                                                 psy[j])
        SKIP_MM or SKIP_DMA or nc.sync.dma_start(y_dram[md], ym[:])


def _build_program(c_cap, loop_reps=1):
    nc = bacc.Bacc("TRN2", target_bir_lowering=False, debug=False, num_devices=E)
    xe = nc.dram_tensor("xe", [KD, P, c_cap], BF16, kind="ExternalInput").ap()
    xs = nc.dram_tensor("xs", [KD, P, SHARD], BF16, kind="ExternalInput").ap()
    w1s = nc.dram_tensor("w1s", [MH, P, KD * P], BF16, kind="ExternalInput").ap()
    w3s = nc.dram_tensor("w3s", [MH, P, KD * P], BF16, kind="ExternalInput").ap()
    w2s = nc.dram_tensor("w2s", [MD, P, MH * P], BF16, kind="ExternalInput").ap()
    sw1s = nc.dram_tensor("sw1s", [MH, P, KD * P], BF16, kind="ExternalInput").ap()
    sw3s = nc.dram_tensor("sw3s", [MH, P, KD * P], BF16, kind="ExternalInput").ap()
    sw2s = nc.dram_tensor("sw2s", [MD, P, MH * P], BF16, kind="ExternalInput").ap()
    ye = nc.dram_tensor("ye", [MD, P, c_cap], BF16, kind="ExternalOutput").ap()
    ys = nc.dram_tensor("ys", [MD, P, SHARD], BF16, kind="ExternalOutput").ap()

    with tile.TileContext(nc) as tc:
        with tc.tile_pool(name="xpool", bufs=1) as xpool, \
             tc.tile_pool(name="wpool", bufs=3) as wpool, \
             tc.tile_pool(name="w2pool", bufs=3) as w2pool, \
             tc.tile_pool(name="gpool", bufs=1) as gpool, \
             tc.tile_pool(name="spool", bufs=4) as spool, \
             tc.tile_pool(name="ypool", bufs=2) as ypool, \
             tc.tile_pool(name="psum", bufs=8, space="PSUM") as psum:

            def body():
                # xe tiles, chunk0 first so the first matmuls can start
                # after ~3 MB of DMA instead of the full x load.
                xe_tiles = [xpool.tile([P, c_cap], BF16, name=f"x{k}")
                            for k in range(KD)]
                chunks = _col_chunks(c_cap)
                c0, cw = chunks[0]
                for k in range(KD):
                    SKIP_DMA or nc.sync.dma_start(
                        xe_tiles[k][:, c0:c0 + cw], xe[k, :, c0:c0 + cw])
                ge_tiles = [gpool.tile([P, c_cap], BF16, name=f"g{m}")
                            for m in range(MH)]
                for c0, cw in chunks[1:]:
                    for k in range(KD):
                        SKIP_DMA or nc.sync.dma_start(
                            xe_tiles[k][:, c0:c0 + cw], xe[k, :, c0:c0 + cw])
                _emit_stage1(nc, (wpool, spool, psum), xe_tiles, w1s, w3s,
                             ge_tiles, c_cap)
                _emit_stage2(nc, (w2pool, psum), ge_tiles, w2s, ye, ypool,
                             c_cap)
                # shared expert: xs/gs reuse the xe/ge SBUF slots (same tags)
                xs_tiles = [xpool.tile([P, SHARD], BF16, name=f"x{k}")
                            for k in range(KD)]
                for k in range(KD):
                    SKIP_DMA or nc.sync.dma_start(xs_tiles[k][:], xs[k])
                gs_tiles = [gpool.tile([P, SHARD], BF16, name=f"g{m}")
                            for m in range(MH)]
                _emit_stage1(nc, (wpool, spool, psum), xs_tiles, sw1s, sw3s,
                             gs_tiles, SHARD)
                _emit_stage2(nc, (w2pool, psum), gs_tiles, sw2s, ys, ypool,
                             SHARD)

            if loop_reps > 1:
                with tc.For_i(0, loop_reps, 1):
                    body()
            else:
                body()
    nc.compile()
    return nc


def _tile_w13(w):
    # [H, D] -> [MH, P, KD*P] with slab[m, p, k*P+j] = w[m*P+j, k*P+p]
    return np.ascontiguousarray(
        w.reshape(MH, P, KD, P).transpose(0, 3, 2, 1).reshape(MH, P, KD * P)
        .astype(NP_BF16)
    )


def _tile_w2(w):
    # [D, H] -> [MD, P, MH*P] with slab[md, p, kh*P+j] = w[md*P+j, kh*P+p]
    return np.ascontiguousarray(
        w.reshape(MD, P, MH, P).transpose(0, 3, 2, 1).reshape(MD, P, MH * P)
        .astype(NP_BF16)
    )


def _tile_x(xt):
    # [T, D] -> [KD, P, T]
    T = xt.shape[0]
    return np.ascontiguousarray(
        xt.reshape(T, KD, P).transpose(1, 2, 0).astype(NP_BF16))


def _untile_y(y):
    # [MD, P, T] -> [T, D]
    return y.transpose(2, 0, 1).reshape(y.shape[2], D).astype(np.float32)


def prepare(x, gate_w, expert_bias, w1, w2, w3, sw1, sw2, sw3):
    """Host routing + input prep. Returns (nc, in_maps, meta)."""
    x = np.ascontiguousarray(np.asarray(x, dtype=np.float32))
    gate_w = np.asarray(gate_w, dtype=np.float32)
    expert_bias = np.asarray(expert_bias, dtype=np.float32)
    w1 = np.asarray(w1, dtype=np.float32)
    w2 = np.asarray(w2, dtype=np.float32)
    w3 = np.asarray(w3, dtype=np.float32)
    sw1 = np.asarray(sw1, dtype=np.float32)
    sw2 = np.asarray(sw2, dtype=np.float32)
    sw3 = np.asarray(sw3, dtype=np.float32)

    # ---- host router (fp32, matches reference numerics) ----
    logits = x @ gate_w.T  # [N, E] f32
    scores = np.where(
        logits >= 0,
        1.0 / (1.0 + np.exp(-logits, dtype=np.float32)),
        np.exp(logits, dtype=np.float32) / (1.0 + np.exp(logits, dtype=np.float32)),
    ).astype(np.float32)
    biased = scores + expert_bias[None, :]
    i1 = np.argmax(biased, axis=1)
    tmp = biased.copy()
    tmp[np.arange(N_TOK), i1] = -np.inf
    i2 = np.argmax(tmp, axis=1)
    s1 = scores[np.arange(N_TOK), i1]
    s2 = scores[np.arange(N_TOK), i2]
    denom = s1 + s2 + np.float32(1e-20)
    c1 = (s1 / denom * np.float32(ROUTE_SCALE)).astype(np.float32)
    c2 = (s2 / denom * np.float32(ROUTE_SCALE)).astype(np.float32)

    idx_list, cw_list = [], []
    for e in range(E):
        m1 = i1 == e
        m2 = i2 == e
        idx = np.concatenate([np.nonzero(m1)[0], np.nonzero(m2)[0]])
        cw = np.concatenate([c1[m1], c2[m2]]).astype(np.float32)
        idx_list.append(idx)
        cw_list.append(cw)
    counts = [len(i) for i in idx_list]
    c_cap = max(512, -(-max(counts) // 64) * 64)

    # ---- build + compile the SPMD program for this capacity ----
    nc = _build_program(c_cap, loop_reps=int(os.environ.get("MOE_LOOP_REPS", "1")))

    # ---- per-core inputs ----
    in_maps = []
    sw1s = _tile_w13(sw1)
    sw3s = _tile_w13(sw3)
    sw2s = _tile_w2(sw2)
    for c in range(E):
        idx = idx_list[c]
        pad = c_cap - len(idx)
        idx_pad = np.concatenate([idx, np.zeros(pad, dtype=idx.dtype)]) if pad else idx
        xe = x[idx_pad]
        in_maps.append({
            "xe": _tile_x(xe),
            "xs": _tile_x(x[c * SHARD:(c + 1) * SHARD]),
            "w1s": _tile_w13(w1[c]),
            "w3s": _tile_w13(w3[c]),
            "w2s": _tile_w2(w2[c]),
            "sw1s": sw1s,
            "sw3s": sw3s,
            "sw2s": sw2s,
        })

    meta = (idx_list, cw_list, counts)
    return nc, in_maps, meta


def combine(meta, results):
    """Scatter-add per-core outputs into the final [N, D] array."""
    idx_list, cw_list, counts = meta
    out = np.zeros((N_TOK, D), dtype=np.float32)
    for c in range(E):
        r = results[c]
        cnt = counts[c]
        if cnt:
            y_tok = _untile_y(r["ye"])[:cnt]
            out[idx_list[c]] += cw_list[c][:, None] * y_tok
        out[c * SHARD:(c + 1) * SHARD] += _untile_y(r["ys"])
    return out


def kernel(x, gate_w, expert_bias, w1, w2, w3, sw1, sw2, sw3):
    nc, in_maps, meta = prepare(x, gate_w, expert_bias, w1, w2, w3, sw1, sw2, sw3)
    global LAST_RESULTS
    res = run_bass_kernel_spmd(nc, in_maps, core_ids=list(range(E)))
    LAST_RESULTS = res
    return combine(meta, res.results)


# revision 6
# speedup vs baseline: 8.4745x; 7.0517x over previous
"""MoE (top-2 of 8 experts + shared expert) Trainium2 Bass kernel.

Strategy (expert-parallel, host-prepped routing):
  - Router (sigmoid gate + top-2) runs on the host in fp32 (it is tiny);
    it produces per-expert token gather lists and combine weights.
  - Core c computes expert c's SwiGLU FFN densely over the tokens routed to
    it (padded to the max per-expert count, 64-rounded), plus the shared
    expert FFN over the token shard [c*1024, (c+1)*1024).
  - Host scatter-adds the per-expert outputs (scaled by combine weights)
    and the shared outputs into the final [8192, 2048] result.

Device-side numerics: all matmul operands are bf16 (halves HBM traffic vs
f32/f32r at the same 1 cycle/row PE rate); accumulation is fp32 in PSUM.
Measured end-to-end rel err ~1e-3 vs the 2e-2 gate.

Schedule: feature-major layout ("K on partitions"), weight tiles are the
OUTER loop so each weight byte is DMA'd exactly once; all x tiles stay
resident in SBUF (bf16 makes them fit) and the token dimension is walked
in column chunks sized so each PSUM accumulator is <=512 fp32 (one bank):
  stage1:  ps1/ps3[m, cols] = sum_k w1T/w3T[k, m].T @ xT[k, cols]
  g = silu(ps1) * ps3                      (ACT + DVE, cast to bf16)
  stage2:  yT[md, cols] = sum_kh w2T[kh, md].T @ gT[kh, cols]
"""

import os
import sys

for _p in ("/opt/trn_rl_repo", "/root/.axon_site/_ro/trn_rl_repo"):
    if os.path.isdir(_p) and _p not in sys.path:
        sys.path.insert(0, _p)

import ml_dtypes
import numpy as np

import concourse.bass as bass  # noqa: F401
import concourse.mybir as mybir
import concourse.tile as tile
from concourse import bacc
from concourse.bass_utils import run_bass_kernel_spmd

# Problem constants (hardcoded per spec)
N_TOK = 8192
D = 2048
H = 1408
E = 8
TOP_K = 2
ROUTE_SCALE = 1.0
P = 128
KD = D // P    # 16 k-tiles over D
MH = H // P    # 11 m-tiles over H
MD = D // P    # 16 m-tiles over D (stage 2 out)
SHARD = N_TOK // E  # 1024 shared-expert tokens per core

F32 = mybir.dt.float32
BF16 = mybir.dt.bfloat16
NP_BF16 = ml_dtypes.bfloat16
SILU = mybir.ActivationFunctionType.Silu

LAST_RESULTS = None  # BassKernelResults of the most recent run (for test.py)

SKIP_MM = bool(os.environ.get("MOE_SKIP_MM"))
SKIP_DMA = bool(os.environ.get("MOE_SKIP_DMA"))
# MOE_PROBE=nodma: replace every input DMA with a gpsimd memset and skip
# output DMAs — isolates the PE/ACT/DVE schedule for timing attribution.
PROBE_NODMA = os.environ.get("MOE_PROBE") == "nodma"


def _load(nc, tile_ap, dram_ap):
    if PROBE_NODMA:
        nc.gpsimd.memset(tile_ap, 0.01)
    elif not SKIP_DMA:
        nc.sync.dma_start(tile_ap, dram_ap)


def _col_chunks(T, lead=0):
    """Split T (multiple of 64) into even chunks of <=1024 cols (<=2 PSUM
    banks per accumulator side), multiples of 64. A small `lead` chunk lets
    the first matmul start after ~1 MB of x DMA instead of ~3 MB."""
    out = []
    c0 = 0
    if lead and T > lead:
        out.append((0, lead))
        c0 = lead
    rem = T - c0
    n = -(-rem // 1024)
    c = (-(-rem // n) + 63) // 64 * 64
    while c0 < T:
        cw = min(c, T - c0)
        out.append((c0, cw))
        c0 += cw
    return out


def _subs_of(cw):
    """Split a chunk into <=512-col PSUM sub-tiles."""
    out = []
    s0 = 0
    while s0 < cw:
        sl = min(512, cw - s0)
        out.append((s0, sl))
        s0 += sl
    return out


def _emit_stage1(nc, pools, x_tiles, w1_dram, w3_dram, g_tiles, T, lead=0):
    """g[m] = silu(x @ w1.T) * (x @ w3.T), feature-major, weights outer."""
    wpool, spool, psum = pools
    chunks = _col_chunks(T, lead)
    for m in range(MH):
        w1m = wpool.tile([P, KD * P], BF16, name="w1m")
        _load(nc, w1m[:], w1_dram[m])
        w3m = wpool.tile([P, KD * P], BF16, name="w3m")
        _load(nc, w3m[:], w3_dram[m])
        for c0, cw in chunks:
            subs = _subs_of(cw)
            ps1 = [psum.tile([P, 512], F32, name="acc")[:, :sl] for _, sl in subs]
            ps3 = [psum.tile([P, 512], F32, name="acc")[:, :sl] for _, sl in subs]
            for k in range(KD):
                for j, (s0, sl) in enumerate(subs):
                    SKIP_MM or nc.tensor.matmul(
                        ps1[j], w1m[:, k * P:(k + 1) * P],
                        x_tiles[k][:, c0 + s0:c0 + s0 + sl],
                        start=(k == 0), stop=(k == KD - 1),
                    )
                for j, (s0, sl) in enumerate(subs):
                    SKIP_MM or nc.tensor.matmul(
                        ps3[j], w3m[:, k * P:(k + 1) * P],
                        x_tiles[k][:, c0 + s0:c0 + s0 + sl],
                        start=(k == 0), stop=(k == KD - 1),
                    )
            for j, (s0, sl) in enumerate(subs):
                st = spool.tile([P, 512], F32, name="silu")[:, :sl]
                SKIP_MM or nc.scalar.activation(st, ps1[j], SILU)
                SKIP_MM or nc.vector.tensor_mul(
                    g_tiles[m][:, c0 + s0:c0 + s0 + sl], st, ps3[j])


def _emit_stage2(nc, pools, g_tiles, w2_dram, y_dram, ypool, T):
    """y[md] = g @ w2.T, feature-major, weights outer."""
    w2pool, psum = pools
    chunks = _col_chunks(T)
    for md in range(MD):
        w2m = w2pool.tile([P, MH * P], BF16, name="w2m")
        _load(nc, w2m[:], w2_dram[md])
        ym = ypool.tile([P, T], BF16, name="ym")
        for c0, cw in chunks:
            subs = _subs_of(cw)
            psy = [psum.tile([P, 512], F32, name="acc")[:, :sl] for _, sl in subs]
            for kh in range(MH):
                for j, (s0, sl) in enumerate(subs):
                    SKIP_MM or nc.tensor.matmul(
                        psy[j], w2m[:, kh * P:(kh + 1) * P],
                        g_tiles[kh][:, c0 + s0:c0 + s0 + sl],
                        start=(kh == 0), stop=(kh == MH - 1),
                    )
            for j, (s0, sl) in enumerate(subs):
                SKIP_MM or nc.vector.tensor_copy(ym[:, c0 + s0:c0 + s0 + sl],
                                                 psy[j])
        if not (SKIP_MM or SKIP_DMA or PROBE_NODMA):
            nc.sync.dma_start(y_dram[md], ym[:])


def _build_program(c_cap, loop_reps=1):
    nc = bacc.Bacc("TRN2", target_bir_lowering=False, debug=False, num_devices=E)
    xe = nc.dram_tensor("xe", [KD, P, c_cap], BF16, kind="ExternalInput").ap()
    xs = nc.dram_tensor("xs", [KD, P, SHARD], BF16, kind="ExternalInput").ap()
    w1s = nc.dram_tensor("w1s", [MH, P, KD * P], BF16, kind="ExternalInput").ap()
    w3s = nc.dram_tensor("w3s", [MH, P, KD * P], BF16, kind="ExternalInput").ap()
    w2s = nc.dram_tensor("w2s", [MD, P, MH * P], BF16, kind="ExternalInput").ap()
    sw1s = nc.dram_tensor("sw1s", [MH, P, KD * P], BF16, kind="ExternalInput").ap()
    sw3s = nc.dram_tensor("sw3s", [MH, P, KD * P], BF16, kind="ExternalInput").ap()
    sw2s = nc.dram_tensor("sw2s", [MD, P, MH * P], BF16, kind="ExternalInput").ap()
    ye = nc.dram_tensor("ye", [MD, P, c_cap], BF16, kind="ExternalOutput").ap()
    ys = nc.dram_tensor("ys", [MD, P, SHARD], BF16, kind="ExternalOutput").ap()

    with tile.TileContext(nc) as tc:
        with tc.tile_pool(name="xpool", bufs=1) as xpool, \
             tc.tile_pool(name="wpool", bufs=3) as wpool, \
             tc.tile_pool(name="w2pool", bufs=3) as w2pool, \
             tc.tile_pool(name="gpool", bufs=1) as gpool, \
             tc.tile_pool(name="spool", bufs=4) as spool, \
             tc.tile_pool(name="ypool", bufs=2) as ypool, \
             tc.tile_pool(name="psum", bufs=8, space="PSUM") as psum:

            def body():
                # xe tiles, chunk0 first so the first matmuls can start
                # after ~3 MB of DMA instead of the full x load.
                xe_tiles = [xpool.tile([P, c_cap], BF16, name=f"x{k}")
                            for k in range(KD)]
                chunks = _col_chunks(c_cap, lead=256)
                c0, cw = chunks[0]
                for k in range(KD):
                    _load(nc, xe_tiles[k][:, c0:c0 + cw], xe[k, :, c0:c0 + cw])
                ge_tiles = [gpool.tile([P, c_cap], BF16, name=f"g{m}")
                            for m in range(MH)]
                for c0, cw in chunks[1:]:
                    for k in range(KD):
                        _load(nc, xe_tiles[k][:, c0:c0 + cw],
                              xe[k, :, c0:c0 + cw])
                _emit_stage1(nc, (wpool, spool, psum), xe_tiles, w1s, w3s,
                             ge_tiles, c_cap, lead=256)
                _emit_stage2(nc, (w2pool, psum), ge_tiles, w2s, ye, ypool,
                             c_cap)
                # shared expert: xs/gs reuse the xe/ge SBUF slots (same tags)
                xs_tiles = [xpool.tile([P, SHARD], BF16, name=f"x{k}")
                            for k in range(KD)]
                for k in range(KD):
                    _load(nc, xs_tiles[k][:], xs[k])
                gs_tiles = [gpool.tile([P, SHARD], BF16, name=f"g{m}")
                            for m in range(MH)]
                _emit_stage1(nc, (wpool, spool, psum), xs_tiles, sw1s, sw3s,
                             gs_tiles, SHARD)
                _emit_stage2(nc, (w2pool, psum), gs_tiles, sw2s, ys, ypool,
                             SHARD)

            if loop_reps > 1:
                with tc.For_i(0, loop_reps, 1):
                    body()
            else:
                body()
    nc.compile()
    return nc


def _tile_w13(w):
    # [H, D] -> [MH, P, KD*P] with slab[m, p, k*P+j] = w[m*P+j, k*P+p]
    return np.ascontiguousarray(
        w.reshape(MH, P, KD, P).transpose(0, 3, 2, 1).reshape(MH, P, KD * P)
        .astype(NP_BF16)
    )


def _tile_w2(w):
    # [D, H] -> [MD, P, MH*P] with slab[md, p, kh*P+j] = w[md*P+j, kh*P+p]
    return np.ascontiguousarray(
        w.reshape(MD, P, MH, P).transpose(0, 3, 2, 1).reshape(MD, P, MH * P)
        .astype(NP_BF16)
    )


def _tile_x(xt):
    # [T, D] -> [KD, P, T]
    T = xt.shape[0]
    return np.ascontiguousarray(
        xt.reshape(T, KD, P).transpose(1, 2, 0).astype(NP_BF16))


def _untile_y(y):
    # [MD, P, T] -> [T, D]
    return y.transpose(2, 0, 1).reshape(y.shape[2], D).astype(np.float32)


def prepare(x, gate_w, expert_bias, w1, w2, w3, sw1, sw2, sw3):
    """Host routing + input prep. Returns (nc, in_maps, meta)."""
    x = np.ascontiguousarray(np.asarray(x, dtype=np.float32))
    gate_w = np.asarray(gate_w, dtype=np.float32)
    expert_bias = np.asarray(expert_bias, dtype=np.float32)
    w1 = np.asarray(w1, dtype=np.float32)
    w2 = np.asarray(w2, dtype=np.float32)
    w3 = np.asarray(w3, dtype=np.float32)
    sw1 = np.asarray(sw1, dtype=np.float32)
    sw2 = np.asarray(sw2, dtype=np.float32)
    sw3 = np.asarray(sw3, dtype=np.float32)

    # ---- host router (fp32, matches reference numerics) ----
    logits = x @ gate_w.T  # [N, E] f32
    scores = np.where(
        logits >= 0,
        1.0 / (1.0 + np.exp(-logits, dtype=np.float32)),
        np.exp(logits, dtype=np.float32) / (1.0 + np.exp(logits, dtype=np.float32)),
    ).astype(np.float32)
    biased = scores + expert_bias[None, :]
    i1 = np.argmax(biased, axis=1)
    tmp = biased.copy()
    tmp[np.arange(N_TOK), i1] = -np.inf
    i2 = np.argmax(tmp, axis=1)
    s1 = scores[np.arange(N_TOK), i1]
    s2 = scores[np.arange(N_TOK), i2]
    denom = s1 + s2 + np.float32(1e-20)
    c1 = (s1 / denom * np.float32(ROUTE_SCALE)).astype(np.float32)
    c2 = (s2 / denom * np.float32(ROUTE_SCALE)).astype(np.float32)

    idx_list, cw_list = [], []
    for e in range(E):
        m1 = i1 == e
        m2 = i2 == e
        idx = np.concatenate([np.nonzero(m1)[0], np.nonzero(m2)[0]])
        cw = np.concatenate([c1[m1], c2[m2]]).astype(np.float32)
        idx_list.append(idx)
        cw_list.append(cw)
    counts = [len(i) for i in idx_list]
    c_cap = max(512, -(-max(counts) // 64) * 64)

    # ---- build + compile the SPMD program for this capacity ----
    nc = _build_program(c_cap, loop_reps=int(os.environ.get("MOE_LOOP_REPS", "1")))

    # ---- per-core inputs ----
    in_maps = []
    sw1s = _tile_w13(sw1)
    sw3s = _tile_w13(sw3)
    sw2s = _tile_w2(sw2)
    for c in range(E):
        idx = idx_list[c]
        pad = c_cap - len(idx)
        idx_pad = np.concatenate([idx, np.zeros(pad, dtype=idx.dtype)]) if pad else idx
        xe = x[idx_pad]
        in_maps.append({
            "xe": _tile_x(xe),
            "xs": _tile_x(x[c * SHARD:(c + 1) * SHARD]),
            "w1s": _tile_w13(w1[c]),
            "w3s": _tile_w13(w3[c]),
            "w2s": _tile_w2(w2[c]),
            "sw1s": sw1s,
            "sw3s": sw3s,
            "sw2s": sw2s,
        })

    meta = (idx_list, cw_list, counts)
    return nc, in_maps, meta


def combine(meta, results):
    """Scatter-add per-core outputs into the final [N, D] array."""
    idx_list, cw_list, counts = meta
    out = np.zeros((N_TOK, D), dtype=np.float32)
    for c in range(E):
        r = results[c]
        cnt = counts[c]
        if cnt:
            y_tok = _untile_y(r["ye"])[:cnt]
            out[idx_list[c]] += cw_list[c][:, None] * y_tok
        out[c * SHARD:(c + 1) * SHARD] += _untile_y(r["ys"])
    return out


def kernel(x, gate_w, expert_bias, w1, w2, w3, sw1, sw2, sw3):
    nc, in_maps, meta = prepare(x, gate_w, expert_bias, w1, w2, w3, sw1, sw2, sw3)
    global LAST_RESULTS
    res = run_bass_kernel_spmd(nc, in_maps, core_ids=list(range(E)))
    LAST_RESULTS = res
    return combine(meta, res.results)
